# revision 17
# baseline (speedup 1.0000x reference)
# BertSelfAttention Trainium2 Bass kernel (v2).
#
# Problem: B=4, S=2048, HID=1024, NH=16, HD=64, fp32.
#   out = softmax((X Wq + bq)(X Wk + bk)^T / sqrt(HD) + mask) (X Wv + bv)
#
# Sharding (8 cores): data-parallel over B (4) x tensor-parallel over the 16
# heads (2 halves of 8 heads = 512 columns of Wq/Wk/Wv). core = b*2 + half.
# No cross-core communication; each core computes attention for its 8 heads
# and writes out[b, :, half*512:(half+1)*512].
#
# v2 changes vs v1:
#   - All bulk inputs loaded with one DMA each (xt / wq / wk / wv); biases
#     and mask packed host-side into one [128, 24] tensor (147 -> ~26 DMAs).
#   - Probabilities (exp output) and V stored as bf16: halves ACT/DVE/SBUF
#     cost of the softmax pipeline; ctx matmuls run bf16 x bf16 (full rate).
#   - Tail batched: per (c, q4, hsub) the 4 transposed 128x65 ctx blocks
#     share one PSUM bank, one reciprocal over the 4 denominators, one
#     scalar_tensor_tensor multiply with a stride-0 broadcast, one output
#     DMA per (c, q4) (64 -> 16 output DMAs).
#   bv is added on the host: softmax rows sum to 1, so probs @ (V0 + bv)
#   = probs @ V0 + bv exactly (dropout prob = 0).
#
# No max-subtraction in softmax: exp(s/8 + m) at this problem's scale is far
# inside fp32 range, and large-negative masks underflow to 0 correctly.

import sys

if "/opt/trn_rl_repo" not in sys.path:
    sys.path.insert(0, "/opt/trn_rl_repo")

import numpy as np

P = 128
B, S, HID = 4, 2048, 1024
NH, HD = 16, 64
COLS = 512          # per-core slice of the hidden dim (8 heads)
HC = HID // P       # 8 hid chunks
SEQB = S // P       # 16 seq blocks (also the k blocks)
CC = COLS // P      # 4 col chunks (each = 2 heads)
QT = S // 512       # 4 q tiles of 512
KB = S // P         # 16 k blocks of 128
N_CORES = 8

_prog_cache = {}


def _build_program(repeat=1, ablate=()):
    ablate = set(ablate)
    import concourse.mybir as mybir
    from concourse import bacc
    from concourse.tile import TileContext
    from concourse.masks import make_identity

    dt = mybir.dt
    F32 = dt.float32
    F32R = dt.float32r
    BF16 = dt.bfloat16
    FP16 = dt.float16
    EXP = mybir.ActivationFunctionType.Exp
    ADD = mybir.AluOpType.add
    MULT = mybir.AluOpType.mult

    nc = bacc.Bacc(num_devices=N_CORES)

    x = nc.dram_tensor("x", [HID, S], FP16, kind="ExternalInput")  # X^T (host: fp16)
    wq = nc.dram_tensor("wq", [HID, COLS], FP16, kind="ExternalInput")
    wk = nc.dram_tensor("wk", [HID, COLS], FP16, kind="ExternalInput")
    wv = nc.dram_tensor("wv", [HID, COLS], FP16, kind="ExternalInput")
    # host packs [bq(4) | bk(4) | mask(16)] as [128, 24]:
    #   col c       = bq[c*128 + p],  col CC + c  = bk[c*128 + p],
    #   col 2CC + kb = mask[kb*128 + p]
    bqkm = nc.dram_tensor("bqkm", [P, 2 * CC + KB], F32, kind="ExternalInput")
    out = nc.dram_tensor("out", [S, COLS], F32, kind="ExternalOutput")

    def emit(tc):
        with (
            tc.tile_pool(name="persist", bufs=1) as persist,
            tc.tile_pool(name="ps_proj", bufs=1, space="PSUM") as ps_proj,
        ):
            ident = persist.tile([P, P], F32)
            make_identity(nc, ident[:])

            bqkm_t = persist.tile([P, 2 * CC + KB], F32, tag="bqkm")
            nc.sync.dma_start(bqkm_t[:], bqkm[:])
            bq_col = lambda c: bqkm_t[:, c:c + 1]
            bk_col = lambda c: bqkm_t[:, CC + c:CC + c + 1]
            mask_col = lambda kb: bqkm_t[:, 2 * CC + kb:2 * CC + kb + 1]

            # XT[p, hc, s] = x[hc*128 + p, s]; one DMA per hc chunk so the
            # first projection matmul (which consumes hc sequentially) can
            # start as soon as chunk 0 lands.
            xt = persist.tile([P, HC, S], FP16, tag="xt")
            x_r = x[:, :].rearrange("(hc p) s -> p hc s", p=P)
            for hc in range(HC):
                nc.sync.dma_start(xt[:, hc, :], x_r[:, hc, :])

            # weights resident in SBUF: w*_t[p, hc, c] = w*[hc*128 + p, c];
            # wq/wk split by column half so proj c=0/1 isn't gated on the
            # full megabyte.
            wq_t = persist.tile([P, HC, COLS], FP16, tag="wq")
            wk_t = persist.tile([P, HC, COLS], FP16, tag="wk")
            wv_t = persist.tile([P, HC, COLS], FP16, tag="wv")
            wq_r = wq[:, :].rearrange("(hc p) c -> p hc c", p=P)
            wk_r = wk[:, :].rearrange("(hc p) c -> p hc c", p=P)
            nc.sync.dma_start(wq_t[:, :, 0:COLS // 2], wq_r[:, :, 0:COLS // 2])
            nc.sync.dma_start(wk_t[:, :, 0:COLS // 2], wk_r[:, :, 0:COLS // 2])
            nc.sync.dma_start(wq_t[:, :, COLS // 2:], wq_r[:, :, COLS // 2:])
            nc.sync.dma_start(wk_t[:, :, COLS // 2:], wk_r[:, :, COLS // 2:])
            nc.sync.dma_start(
                wv_t[:], wv[:, :].rearrange("(hc p) c -> p hc c", p=P))

            # v_t[p, kb, h, 0:64] = V[kb*128 + p, h*64 + d] (bf16);
            # v_t[..., 64] = 1 so the ctx matmul also yields the softmax
            # denominator.
            v_t = persist.tile([P, KB, 8, HD + 1], BF16, tag="v")
            ones_t = persist.tile([P, 1], F32, tag="ones")
            nc.gpsimd.memset(ones_t[:], 1.0)
            nc.vector.tensor_copy(
                out=v_t[:, :, :, HD],
                in_=ones_t[:, 0, None, None].to_broadcast([P, KB, 8]),
            )

            with (
                tc.tile_pool(name="qkpool", bufs=2) as qkpool,
                tc.tile_pool(name="exps", bufs=6) as exps_pool,
                tc.tile_pool(name="small", bufs=2) as small,
                tc.tile_pool(name="ps_sc", bufs=2, space="PSUM") as ps_sc,
                tc.tile_pool(name="ps_ctx", bufs=1, space="PSUM") as ps_ctx,
                tc.tile_pool(name="ps_ctr", bufs=1, space="PSUM") as ps_ctr,
            ):
                def qk_proj(c, qt_t, kt_t):
                    for s4 in range(QT):
                        sl = slice(s4 * 512, (s4 + 1) * 512)
                        psq = ps_proj.tile([P, 512], F32, tag="proj",
                                           name=f"psq_{c}_{s4}")
                        for hc in range(HC):
                            nc.tensor.matmul(
                                psq[:], wq_t[:, hc, c * P:(c + 1) * P],
                                xt[:, hc, sl],
                                start=(hc == 0), stop=(hc == HC - 1),
                            )
                        nc.vector.tensor_scalar(
                            qt_t[:, sl], psq[:], bq_col(c), None, ADD
                        )
                        psk = ps_proj.tile([P, 512], F32, tag="proj",
                                           name=f"psk_{c}_{s4}")
                        for hc in range(HC):
                            nc.tensor.matmul(
                                psk[:], wk_t[:, hc, c * P:(c + 1) * P],
                                xt[:, hc, sl],
                                start=(hc == 0), stop=(hc == HC - 1),
                            )
                        nc.vector.tensor_scalar(
                            kt_t[:, sl], psk[:], bk_col(c), None, ADD
                        )

                def v_proj_sb(sb):
                    psv = ps_proj.tile([P, COLS], F32, tag="proj",
                                       name=f"psv_{sb}")
                    for hc in range(HC):
                        nc.tensor.matmul(
                            psv[:],
                            xt[:, hc, sb * P:(sb + 1) * P],
                            wv_t[:, hc, :],
                            start=(hc == 0), stop=(hc == HC - 1),
                        )
                    nc.vector.tensor_copy(
                        out=v_t[:, sb, :, 0:HD],
                        in_=psv[:].rearrange("p (h d) -> p h d", d=HD),
                    )

                def attention(c, qt_t, kt_t):
                    # heads (2c, 2c+1); hsub 0 -> partitions 0:64, hsub 1 ->
                    # 64:128 (concurrent PE row groups). ctx matmuls are
                    # software-pipelined one k-block behind the score matmuls
                    # so PE always has ready work while ACT runs exp.
                    for q4 in range(QT):
                        qsl = slice(q4 * 512, (q4 + 1) * 512)
                        psc = [
                            ps_ctx.tile([HD + 1, 512], F32, tag=f"ctx{h}",
                                        name=f"psc_{c}_{q4}_{h}")
                            for h in range(2)
                        ]
                        exp_tiles = []

                        def ctx_mm(j):
                            if "ctx" in ablate:
                                return
                            for hsub in range(2):
                                esrc = 0 if "exphalf" in ablate else hsub
                                nc.tensor.matmul(
                                    psc[hsub][:],
                                    v_t[:, j, 2 * c + hsub, :],
                                    exp_tiles[j][:, esrc, :],
                                    start=(j == 0), stop=(j == KB - 1),
                                )

                        for kb in range(KB):
                            ksl = slice(kb * P, (kb + 1) * P)
                            pss = ps_sc.tile([P, 2, 512], F32, tag="sc",
                                             name=f"pss_{c}_{q4}_{kb}")
                            if "scores" not in ablate:
                                for hsub in range(2):
                                    hp = slice(hsub * HD, hsub * HD + HD)
                                    nc.tensor.matmul(
                                        pss[:, hsub, :],
                                        kt_t[hp, ksl],
                                        qt_t[hp, qsl],
                                        start=True, stop=True,
                                    )
                            et = exps_pool.tile([P, 2, 512], BF16, tag="e",
                                                name=f"et_{c}_{q4}_{kb}")
                            if "exphalf" in ablate:
                                # timing probe: half the ACT work (wrong
                                # results, same PE/DMA structure; ctx hsub1
                                # reads hsub0's probs)
                                nc.scalar.activation(
                                    et[:, 0, :], pss[:, 0, :], EXP,
                                    bias=mask_col(kb), scale=0.125,
                                )
                            elif "exp" not in ablate:
                                # exp(s/8 + mask_k); mask = per-partition bias
                                nc.scalar.activation(
                                    et[:], pss[:], EXP,
                                    bias=mask_col(kb), scale=0.125,
                                )
                            exp_tiles.append(et)
                            if c == 0 and q4 == 0:
                                v_proj_sb(kb)
                            if kb > 0:
                                ctx_mm(kb - 1)
                        ctx_mm(KB - 1)

                        if "tail" in ablate:
                            continue
                        # ev[p, qb, hsub, d] = out[q4*512 + qb*128 + p,
                        #                          c*128 + hsub*64 + d]
                        ev = small.tile([P, QT, 2, HD], F32, tag="ev",
                                        name=f"ev_{c}_{q4}")
                        for hsub in range(2):
                            ctxt = small.tile([HD + 1, 512], F32,
                                              tag=f"ct{hsub}",
                                              name=f"ctxt_{c}_{q4}_{hsub}")
                            nc.vector.tensor_copy(out=ctxt[:], in_=psc[hsub][:])
                            pstr = ps_ctr.tile([P, QT, HD + 1], F32,
                                               tag="ctr",
                                               name=f"pstr_{c}_{q4}_{hsub}")
                            for qb in range(QT):
                                nc.tensor.transpose(
                                    pstr[:, qb, :],
                                    ctxt[:, qb * P:(qb + 1) * P],
                                    ident[0:HD + 1, 0:HD + 1],
                                )
                            rec = small.tile([P, QT], F32, tag="rec",
                                             bufs=2,
                                             name=f"rec_{c}_{q4}_{hsub}")
                            nc.vector.reciprocal(rec[:], pstr[:, :, HD])
                            nc.vector.scalar_tensor_tensor(
                                ev[:, :, hsub, :],
                                pstr[:, :, 0:HD],
                                1.0,
                                rec[:, :, None].to_broadcast([P, QT, HD]),
                                MULT, MULT,
                            )
                        nc.sync.dma_start(
                            out[q4 * 512:(q4 + 1) * 512, c * P:(c + 1) * P]
                            .rearrange("(qb p) (h d) -> p qb h d", p=P, d=HD),
                            ev[:],
                        )

                qk_tiles = {}
                for c in range(CC):
                    qk_tiles[c] = (
                        qkpool.tile([P, S], F32R, tag="qt", name=f"qt_t_{c}"),
                        qkpool.tile([P, S], F32R, tag="kt", name=f"kt_t_{c}"),
                    )
                    qk_proj(c, *qk_tiles[c])
                    attention(c, *qk_tiles[c])

    with TileContext(nc) as tc:
        if repeat > 1:
            hints = (
                mybir.EngineType.PE, mybir.EngineType.Activation,
                mybir.EngineType.DVE, mybir.EngineType.SP,
                mybir.EngineType.Pool,
            )
            with tc.For_i(0, repeat, 1, hint_engines=hints,
                          staggered_reset=True):
                emit(tc)
        else:
            emit(tc)
    nc.compile()
    return nc


def _get_program():
    if "nc" not in _prog_cache:
        _prog_cache["nc"] = _build_program()
    return _prog_cache["nc"]


def make_in_maps(hidden_states, attention_mask, Wq, bq, Wk, bk, Wv):
    in_maps = []
    for core in range(N_CORES):
        b, half = core // 2, core % 2
        csl = slice(half * COLS, (half + 1) * COLS)
        bqkm = np.concatenate(
            [
                np.asarray(bq[csl], dtype=np.float32).reshape(CC, P).T,
                np.asarray(bk[csl], dtype=np.float32).reshape(CC, P).T,
                np.asarray(attention_mask[b, 0, 0, :], dtype=np.float32)
                .reshape(KB, P).T,
            ],
            axis=1,
        )
        in_maps.append({
            "x": np.ascontiguousarray(hidden_states[b].T.astype(np.float16)),
            "wq": np.ascontiguousarray(Wq[:, csl].astype(np.float16)),
            "wk": np.ascontiguousarray(Wk[:, csl].astype(np.float16)),
            "wv": np.ascontiguousarray(Wv[:, csl].astype(np.float16)),
            "bqkm": np.ascontiguousarray(bqkm),
        })
    return in_maps


def assemble_output(core_outs, bv):
    full = np.empty((B, S, HID), dtype=np.float32)
    for core in range(N_CORES):
        b, half = core // 2, core % 2
        full[b, :, half * COLS:(half + 1) * COLS] = core_outs[core]
    # exact bv handling: probs rows sum to 1 -> probs @ (V + bv) = ctx + bv
    full += np.asarray(bv, dtype=np.float32).reshape(1, 1, HID)
    return full


def kernel(hidden_states, attention_mask, Wq, bq, Wk, bk, Wv, bv):
    from concourse.bass_utils import run_bass_kernel_spmd

    hidden_states = np.asarray(hidden_states, dtype=np.float32)
    attention_mask = np.asarray(attention_mask, dtype=np.float32)
    Wq = np.asarray(Wq, dtype=np.float32)
    Wk = np.asarray(Wk, dtype=np.float32)
    Wv = np.asarray(Wv, dtype=np.float32)
    bq = np.asarray(bq, dtype=np.float32)
    bk = np.asarray(bk, dtype=np.float32)
    bv = np.asarray(bv, dtype=np.float32)

    nc = _get_program()
    in_maps = make_in_maps(hidden_states, attention_mask, Wq, bq, Wk, bk, Wv)
    res = run_bass_kernel_spmd(nc, in_maps, list(range(N_CORES)))
    return assemble_output([res.results[i]["out"] for i in range(N_CORES)], bv)


# revision 18
# speedup vs baseline: 1.1169x; 1.1169x over previous
# BertSelfAttention Trainium2 Bass kernel (v2).
#
# Problem: B=4, S=2048, HID=1024, NH=16, HD=64, fp32.
#   out = softmax((X Wq + bq)(X Wk + bk)^T / sqrt(HD) + mask) (X Wv + bv)
#
# Sharding (8 cores): data-parallel over B (4) x tensor-parallel over the 16
# heads (2 halves of 8 heads = 512 columns of Wq/Wk/Wv). core = b*2 + half.
# No cross-core communication; each core computes attention for its 8 heads
# and writes out[b, :, half*512:(half+1)*512].
#
# v2 changes vs v1:
#   - All bulk inputs loaded with one DMA each (xt / wq / wk / wv); biases
#     and mask packed host-side into one [128, 24] tensor (147 -> ~26 DMAs).
#   - Probabilities (exp output) and V stored as bf16: halves ACT/DVE/SBUF
#     cost of the softmax pipeline; ctx matmuls run bf16 x bf16 (full rate).
#   - Tail batched: per (c, q4, hsub) the 4 transposed 128x65 ctx blocks
#     share one PSUM bank, one reciprocal over the 4 denominators, one
#     scalar_tensor_tensor multiply with a stride-0 broadcast, one output
#     DMA per (c, q4) (64 -> 16 output DMAs).
#   bv is added on the host: softmax rows sum to 1, so probs @ (V0 + bv)
#   = probs @ V0 + bv exactly (dropout prob = 0).
#
# No max-subtraction in softmax: exp(s/8 + m) at this problem's scale is far
# inside fp32 range, and large-negative masks underflow to 0 correctly.

import sys

if "/opt/trn_rl_repo" not in sys.path:
    sys.path.insert(0, "/opt/trn_rl_repo")

import numpy as np

P = 128
B, S, HID = 4, 2048, 1024
NH, HD = 16, 64
COLS = 512          # per-core slice of the hidden dim (8 heads)
HC = HID // P       # 8 hid chunks
SEQB = S // P       # 16 seq blocks (also the k blocks)
CC = COLS // P      # 4 col chunks (each = 2 heads)
QT = S // 512       # 4 q tiles of 512
KB = S // P         # 16 k blocks of 128
N_CORES = 8

_prog_cache = {}


def _build_program(repeat=1, ablate=()):
    ablate = set(ablate)
    import concourse.mybir as mybir
    from concourse import bacc
    from concourse.tile import TileContext
    from concourse.masks import make_identity

    dt = mybir.dt
    F32 = dt.float32
    F32R = dt.float32r
    BF16 = dt.bfloat16
    FP16 = dt.float16
    EXP = mybir.ActivationFunctionType.Exp
    ADD = mybir.AluOpType.add
    MULT = mybir.AluOpType.mult

    nc = bacc.Bacc(num_devices=N_CORES)

    x = nc.dram_tensor("x", [HID, S], FP16, kind="ExternalInput")  # X^T (host: fp16)
    wq = nc.dram_tensor("wq", [HID, COLS], FP16, kind="ExternalInput")
    wk = nc.dram_tensor("wk", [HID, COLS], FP16, kind="ExternalInput")
    wv = nc.dram_tensor("wv", [HID, COLS], FP16, kind="ExternalInput")
    # host packs [bq(4) | bk(4) | mask(16)] as [128, 24]:
    #   col c       = bq[c*128 + p],  col CC + c  = bk[c*128 + p],
    #   col 2CC + kb = mask[kb*128 + p]
    bqkm = nc.dram_tensor("bqkm", [P, 2 * CC + KB], F32, kind="ExternalInput")
    out = nc.dram_tensor("out", [S, COLS], F32, kind="ExternalOutput")

    def emit(tc):
        with (
            tc.tile_pool(name="persist", bufs=1) as persist,
            tc.tile_pool(name="ps_proj", bufs=1, space="PSUM") as ps_proj,
        ):
            ident = persist.tile([P, P], F32)
            make_identity(nc, ident[:])

            bqkm_t = persist.tile([P, 2 * CC + KB], F32, tag="bqkm")
            nc.sync.dma_start(bqkm_t[:], bqkm[:])
            bq_col = lambda c: bqkm_t[:, c:c + 1]
            bk_col = lambda c: bqkm_t[:, CC + c:CC + c + 1]
            mask_col = lambda kb: bqkm_t[:, 2 * CC + kb:2 * CC + kb + 1]

            # XT[p, hc, s] = x[hc*128 + p, s]; one DMA per hc chunk so the
            # first projection matmul (which consumes hc sequentially) can
            # start as soon as chunk 0 lands.
            xt = persist.tile([P, HC, S], FP16, tag="xt")
            x_r = x[:, :].rearrange("(hc p) s -> p hc s", p=P)
            for hc in range(HC):
                nc.sync.dma_start(xt[:, hc, :], x_r[:, hc, :])

            # weights resident in SBUF: w*_t[p, hc, c] = w*[hc*128 + p, c];
            # wq/wk split by column half so proj c=0/1 isn't gated on the
            # full megabyte.
            wq_t = persist.tile([P, HC, COLS], FP16, tag="wq")
            wk_t = persist.tile([P, HC, COLS], FP16, tag="wk")
            wv_t = persist.tile([P, HC, COLS], FP16, tag="wv")
            wq_r = wq[:, :].rearrange("(hc p) c -> p hc c", p=P)
            wk_r = wk[:, :].rearrange("(hc p) c -> p hc c", p=P)
            nc.sync.dma_start(wq_t[:, :, 0:COLS // 2], wq_r[:, :, 0:COLS // 2])
            nc.sync.dma_start(wk_t[:, :, 0:COLS // 2], wk_r[:, :, 0:COLS // 2])
            nc.sync.dma_start(wq_t[:, :, COLS // 2:], wq_r[:, :, COLS // 2:])
            nc.sync.dma_start(wk_t[:, :, COLS // 2:], wk_r[:, :, COLS // 2:])
            nc.sync.dma_start(
                wv_t[:], wv[:, :].rearrange("(hc p) c -> p hc c", p=P))

            # v_t[p, kb, h, 0:64] = V[kb*128 + p, h*64 + d] (bf16);
            # v_t[..., 64] = 1 so the ctx matmul also yields the softmax
            # denominator.
            v_t = persist.tile([P, KB, 8, HD + 1], BF16, tag="v")
            ones_t = persist.tile([P, 1], F32, tag="ones")
            nc.gpsimd.memset(ones_t[:], 1.0)
            nc.vector.tensor_copy(
                out=v_t[:, :, :, HD],
                in_=ones_t[:, 0, None, None].to_broadcast([P, KB, 8]),
            )

            with (
                tc.tile_pool(name="qkpool", bufs=4) as qkpool,
                tc.tile_pool(name="exps", bufs=6) as exps_pool,
                tc.tile_pool(name="small", bufs=2) as small,
                tc.tile_pool(name="ps_sc", bufs=2, space="PSUM") as ps_sc,
                tc.tile_pool(name="ps_ctx", bufs=1, space="PSUM") as ps_ctx,
                tc.tile_pool(name="ps_ctr", bufs=1, space="PSUM") as ps_ctr,
            ):
                def qk_proj(c, qt_t, kt_t):
                    for s4 in range(QT):
                        sl = slice(s4 * 512, (s4 + 1) * 512)
                        psq = ps_proj.tile([P, 512], F32, tag="proj",
                                           name=f"psq_{c}_{s4}")
                        for hc in range(HC):
                            nc.tensor.matmul(
                                psq[:], wq_t[:, hc, c * P:(c + 1) * P],
                                xt[:, hc, sl],
                                start=(hc == 0), stop=(hc == HC - 1),
                            )
                        nc.vector.tensor_scalar(
                            qt_t[:, sl], psq[:], bq_col(c), None, ADD
                        )
                        psk = ps_proj.tile([P, 512], F32, tag="proj",
                                           name=f"psk_{c}_{s4}")
                        for hc in range(HC):
                            nc.tensor.matmul(
                                psk[:], wk_t[:, hc, c * P:(c + 1) * P],
                                xt[:, hc, sl],
                                start=(hc == 0), stop=(hc == HC - 1),
                            )
                        nc.vector.tensor_scalar(
                            kt_t[:, sl], psk[:], bk_col(c), None, ADD
                        )

                def v_proj_sb(sb):
                    psv = ps_proj.tile([P, COLS], F32, tag="proj",
                                       name=f"psv_{sb}")
                    for hc in range(HC):
                        nc.tensor.matmul(
                            psv[:],
                            xt[:, hc, sb * P:(sb + 1) * P],
                            wv_t[:, hc, :],
                            start=(hc == 0), stop=(hc == HC - 1),
                        )
                    nc.vector.tensor_copy(
                        out=v_t[:, sb, :, 0:HD],
                        in_=psv[:].rearrange("p (h d) -> p h d", d=HD),
                    )

                def attention(c, qt_t, kt_t):
                    # heads (2c, 2c+1); hsub 0 -> partitions 0:64, hsub 1 ->
                    # 64:128 (concurrent PE row groups). ctx matmuls are
                    # software-pipelined one k-block behind the score matmuls
                    # so PE always has ready work while ACT runs exp.
                    for q4 in range(QT):
                        qsl = slice(q4 * 512, (q4 + 1) * 512)
                        psc = [
                            ps_ctx.tile([HD + 1, 512], F32, tag=f"ctx{h}",
                                        name=f"psc_{c}_{q4}_{h}")
                            for h in range(2)
                        ]
                        exp_tiles = []

                        def ctx_mm(j):
                            if "ctx" in ablate:
                                return
                            for hsub in range(2):
                                esrc = 0 if "exphalf" in ablate else hsub
                                nc.tensor.matmul(
                                    psc[hsub][:],
                                    v_t[:, j, 2 * c + hsub, :],
                                    exp_tiles[j][:, esrc, :],
                                    start=(j == 0), stop=(j == KB - 1),
                                )

                        for kb in range(KB):
                            ksl = slice(kb * P, (kb + 1) * P)
                            pss = ps_sc.tile([P, 2, 512], F32, tag="sc",
                                             name=f"pss_{c}_{q4}_{kb}")
                            if "scores" not in ablate:
                                for hsub in range(2):
                                    hp = slice(hsub * HD, hsub * HD + HD)
                                    nc.tensor.matmul(
                                        pss[:, hsub, :],
                                        kt_t[hp, ksl],
                                        qt_t[hp, qsl],
                                        start=True, stop=True,
                                    )
                            et = exps_pool.tile([P, 2, 512], BF16, tag="e",
                                                name=f"et_{c}_{q4}_{kb}")
                            if "exphalf" in ablate:
                                # timing probe: half the ACT work (wrong
                                # results, same PE/DMA structure; ctx hsub1
                                # reads hsub0's probs)
                                nc.scalar.activation(
                                    et[:, 0, :], pss[:, 0, :], EXP,
                                    bias=mask_col(kb), scale=0.125,
                                )
                            elif "exp" not in ablate:
                                # exp(s/8 + mask_k); mask = per-partition bias
                                nc.scalar.activation(
                                    et[:], pss[:], EXP,
                                    bias=mask_col(kb), scale=0.125,
                                )
                            exp_tiles.append(et)
                            if c == 0 and q4 == 0:
                                v_proj_sb(kb)
                            if kb > 0:
                                ctx_mm(kb - 1)
                        ctx_mm(KB - 1)

                        if "tail" in ablate:
                            continue
                        # ev[p, qb, hsub, d] = out[q4*512 + qb*128 + p,
                        #                          c*128 + hsub*64 + d]
                        ev = small.tile([P, QT, 2, HD], F32, tag="ev",
                                        name=f"ev_{c}_{q4}")
                        for hsub in range(2):
                            ctxt = small.tile([HD + 1, 512], F32,
                                              tag=f"ct{hsub}",
                                              name=f"ctxt_{c}_{q4}_{hsub}")
                            nc.vector.tensor_copy(out=ctxt[:], in_=psc[hsub][:])
                            pstr = ps_ctr.tile([P, QT, HD + 1], F32,
                                               tag="ctr",
                                               name=f"pstr_{c}_{q4}_{hsub}")
                            for qb in range(QT):
                                nc.tensor.transpose(
                                    pstr[:, qb, :],
                                    ctxt[:, qb * P:(qb + 1) * P],
                                    ident[0:HD + 1, 0:HD + 1],
                                )
                            rec = small.tile([P, QT], F32, tag="rec",
                                             bufs=2,
                                             name=f"rec_{c}_{q4}_{hsub}")
                            nc.vector.reciprocal(rec[:], pstr[:, :, HD])
                            nc.vector.scalar_tensor_tensor(
                                ev[:, :, hsub, :],
                                pstr[:, :, 0:HD],
                                1.0,
                                rec[:, :, None].to_broadcast([P, QT, HD]),
                                MULT, MULT,
                            )
                        nc.sync.dma_start(
                            out[q4 * 512:(q4 + 1) * 512, c * P:(c + 1) * P]
                            .rearrange("(qb p) (h d) -> p qb h d", p=P, d=HD),
                            ev[:],
                        )

                qk_tiles = {}
                for c in range(CC):
                    qk_tiles[c] = (
                        qkpool.tile([P, S], F32R, tag="qt", name=f"qt_t_{c}"),
                        qkpool.tile([P, S], F32R, tag="kt", name=f"kt_t_{c}"),
                    )
                    qk_proj(c, *qk_tiles[c])
                    attention(c, *qk_tiles[c])

    with TileContext(nc) as tc:
        if repeat > 1:
            hints = (
                mybir.EngineType.PE, mybir.EngineType.Activation,
                mybir.EngineType.DVE, mybir.EngineType.SP,
                mybir.EngineType.Pool,
            )
            with tc.For_i(0, repeat, 1, hint_engines=hints,
                          staggered_reset=True):
                emit(tc)
        else:
            emit(tc)
    nc.compile()
    return nc


def _get_program():
    if "nc" not in _prog_cache:
        _prog_cache["nc"] = _build_program()
    return _prog_cache["nc"]


def make_in_maps(hidden_states, attention_mask, Wq, bq, Wk, bk, Wv):
    in_maps = []
    for core in range(N_CORES):
        b, half = core // 2, core % 2
        csl = slice(half * COLS, (half + 1) * COLS)
        bqkm = np.concatenate(
            [
                np.asarray(bq[csl], dtype=np.float32).reshape(CC, P).T,
                np.asarray(bk[csl], dtype=np.float32).reshape(CC, P).T,
                np.asarray(attention_mask[b, 0, 0, :], dtype=np.float32)
                .reshape(KB, P).T,
            ],
            axis=1,
        )
        in_maps.append({
            "x": np.ascontiguousarray(hidden_states[b].T.astype(np.float16)),
            "wq": np.ascontiguousarray(Wq[:, csl].astype(np.float16)),
            "wk": np.ascontiguousarray(Wk[:, csl].astype(np.float16)),
            "wv": np.ascontiguousarray(Wv[:, csl].astype(np.float16)),
            "bqkm": np.ascontiguousarray(bqkm),
        })
    return in_maps


def assemble_output(core_outs, bv):
    full = np.empty((B, S, HID), dtype=np.float32)
    for core in range(N_CORES):
        b, half = core // 2, core % 2
        full[b, :, half * COLS:(half + 1) * COLS] = core_outs[core]
    # exact bv handling: probs rows sum to 1 -> probs @ (V + bv) = ctx + bv
    full += np.asarray(bv, dtype=np.float32).reshape(1, 1, HID)
    return full


def kernel(hidden_states, attention_mask, Wq, bq, Wk, bk, Wv, bv):
    from concourse.bass_utils import run_bass_kernel_spmd

    hidden_states = np.asarray(hidden_states, dtype=np.float32)
    attention_mask = np.asarray(attention_mask, dtype=np.float32)
    Wq = np.asarray(Wq, dtype=np.float32)
    Wk = np.asarray(Wk, dtype=np.float32)
    Wv = np.asarray(Wv, dtype=np.float32)
    bq = np.asarray(bq, dtype=np.float32)
    bk = np.asarray(bk, dtype=np.float32)
    bv = np.asarray(bv, dtype=np.float32)

    nc = _get_program()
    in_maps = make_in_maps(hidden_states, attention_mask, Wq, bq, Wk, bk, Wv)
    res = run_bass_kernel_spmd(nc, in_maps, list(range(N_CORES)))
    return assemble_output([res.results[i]["out"] for i in range(N_CORES)], bv)
